# revision 1
# baseline (speedup 1.0000x reference)
"""Trainium2 Bass kernel for nn_CrossAttention (b=2, n=1024, dim=1024, h=8,
m=4, cl=1024).

Sharding: 8 cores = batch(2) x head-groups(4, 2 heads each).
Each core computes q/k/v projections for its 2 heads, streaming softmax
attention in transposed layout (S^T = [j, i]), and per-head partial output
projections out^T.  Host sums the 8 partials per batch and adds bout.

Masks are all-ones per the problem spec (fill: ones) so they are ignored.

Layout notes (PE matmul computes lhsT.T @ rhs, lhsT stationary [K<=128, M<=128]):
  qT[dh, i]  = WqT-blocks.T @ xT          (lhsT = WqT [k, dh], rhs = xT [k, i])
  kT[dh, j]  = WkT-blocks.T @ ctxT        (lhsT = WkT [k, dh], rhs = ctxT [k, j])
  v[j, dv]   = ctxT-blocks.T @ WvT        (lhsT = ctxT [k, j], rhs = WvT [k, dv])
  S^T[j, i]  = kT-block.T @ qT            (lhsT = kT [d, j],  rhs = qT [d, i])
  P^T        = exp(scale*S^T + sim[j])    (ScalarE, per-partition bias)
  O^T[dv, i] = sum_j v-block.T @ P^T      (lhsT = v [j, dv],  rhs = P^T [j, i])
  rowsum[1,i]= ones.T @ racc              (racc = sum_j-tiles P^T, DVE+GPSIMD)
  out^T[o,i] = WoutT-block.T @ (O^T/rowsum)
"""

import numpy as np
import ml_dtypes

import concourse.bass as bass
import concourse.tile as tile
import concourse.mybir as mybir
from concourse import bacc
from concourse.bass_utils import run_bass_kernel_spmd

BF16 = mybir.dt.bfloat16
FP32 = mybir.dt.float32
NPBF16 = ml_dtypes.bfloat16

# problem sizes (hardcoded)
B, N, DIM, H, D = 2, 1024, 1024, 8, 128
M, CL = 4, 1024
J = M * CL            # 4096
NHPC = 2              # heads per core
HG = H // NHPC        # head groups = 4
SCALE = DIM ** -0.5   # 1/32
KT = DIM // 128       # 8 k-slices
JT = J // 128         # 32 j-tiles
JQ = 4                # j quarters for kT
P = 128

_CACHE = {}


class _nullctx:
    def __enter__(self):
        return None

    def __exit__(self, *a):
        return False


def _build_nc(reps=1, phases=99):
    nc = bacc.Bacc("TRN2", target_bir_lowering=False, debug=False)

    xT = nc.dram_tensor("xT", [DIM, N], BF16, kind="ExternalInput").ap()
    ctxT = nc.dram_tensor("ctxT", [DIM, J], BF16, kind="ExternalInput").ap()
    wqT = nc.dram_tensor("wqT", [DIM, NHPC * D], BF16, kind="ExternalInput").ap()
    wkT = nc.dram_tensor("wkT", [DIM, NHPC * D], BF16, kind="ExternalInput").ap()
    wvT = nc.dram_tensor("wvT", [DIM, NHPC * D], BF16, kind="ExternalInput").ap()
    woutT = nc.dram_tensor("woutT", [NHPC * D, DIM], BF16, kind="ExternalInput").ap()
    simb = nc.dram_tensor("simb", [P, JT], FP32, kind="ExternalInput").ap()
    pouts = [nc.dram_tensor(f"pout{h}", [DIM, N], FP32, kind="ExternalOutput").ap()
             for h in range(NHPC)]

    with tile.TileContext(nc) as tc:
        with (
            tc.tile_pool(name="const", bufs=1) as constp,
            tc.tile_pool(name="weights", bufs=1) as wp,
            tc.tile_pool(name="acts", bufs=1) as ap_,
            tc.tile_pool(name="work", bufs=1) as workp,
            tc.tile_pool(name="ppool", bufs=3) as ppool,
            tc.tile_pool(name="small", bufs=2) as smallp,
            tc.tile_pool(name="outs", bufs=3) as outp,
            tc.tile_pool(name="psum", bufs=1, space="PSUM") as psum,
        ):
          def _body():
            # ---- constants / weights ----
            ones_col = constp.tile([P, 1], FP32)
            nc.any.memset(ones_col[:], 1.0)
            ones_row = constp.tile([1, P], FP32)
            nc.any.memset(ones_row[:], 1.0)
            simb_s = constp.tile([P, JT], FP32)
            nc.sync.dma_start(simb_s[:], simb[:, :])

            wq_s = wp.tile([P, KT, NHPC * D], BF16)
            nc.sync.dma_start(wq_s[:], wqT.rearrange("(a p) m -> p a m", p=P))
            wk_s = wp.tile([P, KT, NHPC * D], BF16)
            nc.sync.dma_start(wk_s[:], wkT.rearrange("(a p) m -> p a m", p=P))
            wv_s = wp.tile([P, KT, NHPC * D], BF16)
            nc.sync.dma_start(wv_s[:], wvT.rearrange("(a p) m -> p a m", p=P))
            wo_s = wp.tile([P, NHPC, DIM], BF16)
            nc.sync.dma_start(wo_s[:], woutT.rearrange("(a p) m -> p a m", p=P))

            xT_s = ap_.tile([P, KT, N], BF16)
            for k in range(KT):
                nc.sync.dma_start(xT_s[:, k, :], xT[k * P:(k + 1) * P, :])

            # ctx streamed j-quarter-major so kT/v work can start early
            ctx_s = ap_.tile([P, KT, J], BF16)
            JQW = J // JQ  # 1024
            for jq in range(JQ):
                for k in range(KT):
                    nc.sync.dma_start(
                        ctx_s[:, k, jq * JQW:(jq + 1) * JQW],
                        ctxT[k * P:(k + 1) * P, jq * JQW:(jq + 1) * JQW],
                    )

            if phases < 1:
                return
            # ---- phase 1: qT[h] = Wq_h @ x^T  -> [128, N] per head ----
            # projection psum chunks are [128, 512] = 1 bank, tag "pjv" bufs=2
            qT_s = workp.tile([P, NHPC, N], BF16)
            for h in range(NHPC):
                for u in range(N // 512):
                    pq = psum.tile([P, 512], FP32, tag="pjv", bufs=2)
                    for k in range(KT):
                        nc.tensor.matmul(
                            pq[:, :],
                            wq_s[:, k, h * D:(h + 1) * D],
                            xT_s[:, k, u * 512:(u + 1) * 512],
                            start=(k == 0),
                            stop=(k == KT - 1),
                        )
                    nc.vector.tensor_copy(qT_s[:, h, u * 512:(u + 1) * 512], pq[:, :])

            if phases < 2:
                return
            # ---- phase 2+3: kT (both heads) and v, streamed by j-quarter ----
            kT_s = workp.tile([P, NHPC, J], BF16)
            v_s = workp.tile([P, JT, NHPC * D], BF16)

            def kt_chunk(h, e):
                """kT eighth e for head h: [128, 512] psum chunk."""
                pk = psum.tile([P, 512], FP32, tag="pjv", bufs=2)
                for k in range(KT):
                    nc.tensor.matmul(
                        pk[:, :],
                        wk_s[:, k, h * D:(h + 1) * D],
                        ctx_s[:, k, e * 512:(e + 1) * 512],
                        start=(k == 0),
                        stop=(k == KT - 1),
                    )
                nc.vector.tensor_copy(kT_s[:, h, e * 512:(e + 1) * 512], pk[:, :])

            def v_chunk(jt):
                pv = psum.tile([P, NHPC * D], FP32, tag="pjv", bufs=2)
                for k in range(KT):
                    nc.tensor.matmul(
                        pv[:, :],
                        ctx_s[:, k, jt * P:(jt + 1) * P],
                        wv_s[:, k, :],
                        start=(k == 0),
                        stop=(k == KT - 1),
                    )
                nc.scalar.copy(v_s[:, jt, :], pv[:, :])

            # ---- attention building blocks (emitted pipelined below) ----
            def attn_setup():
                pot = psum.tile([P, N], FP32, tag="ot", bufs=1)
                racc_d = smallp.tile([P, N], FP32, tag="racc_d")
                racc_g = smallp.tile([P, N], FP32, tag="racc_g")
                return pot, racc_d, racc_g

            def attn_jt(h, jt, st, fillers):
                pot, racc_d, racc_g = st
                ps_ = psum.tile([P, N], FP32, tag="sc", bufs=2)
                kblk = kT_s[:, h, jt * P:(jt + 1) * P]
                for u in range(N // 512):
                    nc.tensor.matmul(
                        ps_[:, u * 512:(u + 1) * 512],
                        kblk,
                        qT_s[:, h, u * 512:(u + 1) * 512],
                        start=True,
                        stop=True,
                    )
                if fillers:
                    fillers.pop(0)()
                # p = exp(scale * s + sim_j)  (per-partition bias)
                p_t = ppool.tile([P, N], BF16, tag="p")
                nc.scalar.activation(
                    p_t[:], ps_[:, :],
                    mybir.ActivationFunctionType.Exp,
                    bias=simb_s[:, jt:jt + 1],
                    scale=float(SCALE),
                )
                # running softmax denominator, split across DVE and GPSIMD
                if jt == 0:
                    nc.vector.tensor_copy(racc_d[:], p_t[:])
                elif jt == 2:
                    nc.gpsimd.tensor_copy(racc_g[:], p_t[:])
                elif jt % 3 == 2:
                    nc.gpsimd.tensor_add(racc_g[:], racc_g[:], p_t[:])
                else:
                    nc.vector.tensor_add(racc_d[:], racc_d[:], p_t[:])
                # O^T accumulation: lhsT = v block, rhs = p^T tile
                vblk = v_s[:, jt, h * D:(h + 1) * D]
                for u in range(N // 512):
                    nc.tensor.matmul(
                        pot[:, u * 512:(u + 1) * 512],
                        vblk,
                        p_t[:, u * 512:(u + 1) * 512],
                        start=(jt == 0),
                        stop=(jt == JT - 1),
                    )

            # head-0 k/v jq-streamed with head-0 attention pipelined one
            # quarter behind; head-1 kT chunks interleaved as PE filler
            # into the later attention iterations.
            st0 = attn_setup()
            kt1_fillers = [(lambda e=e: kt_chunk(1, e)) for e in range(J // 512)]
            for jq in range(JQ):
                kt_chunk(0, 2 * jq)
                kt_chunk(0, 2 * jq + 1)
                for jt in range(8 * jq, 8 * jq + 8):
                    v_chunk(jt)
                if jq >= 1:
                    for jt in range(8 * (jq - 1), 8 * jq):
                        attn_jt(0, jt, st0, kt1_fillers if jt >= 16 else None)
            for jt in range(24, 32):
                attn_jt(0, jt, st0, kt1_fillers)
            while kt1_fillers:
                kt1_fillers.pop(0)()

            if phases < 4:
                return

            # ---- per head: normalize + out-projection ----
            def attention(h, st=None):
                if st is None:
                    st = attn_setup()
                    for jt in range(JT):
                        attn_jt(h, jt, st, None)
                pot, racc_d, racc_g = st
                nc.vector.tensor_add(racc_d[:], racc_d[:], racc_g[:])
                # rowsum over partitions via ones-matmul -> [1, N]
                prs = psum.tile([P, N], FP32, tag="sc", bufs=2)
                for u in range(N // 512):
                    nc.tensor.matmul(
                        prs[:1, u * 512:(u + 1) * 512],
                        ones_col[:],
                        racc_d[:, u * 512:(u + 1) * 512],
                        start=True, stop=True,
                    )
                recip = smallp.tile([1, N], FP32, tag="recip")
                nc.vector.reciprocal(recip[:], prs[:1, :])
                # broadcast recip to 128 partitions via K=1 matmul
                prb = psum.tile([P, N], FP32, tag="sc", bufs=2)
                for u in range(N // 512):
                    nc.tensor.matmul(
                        prb[:, u * 512:(u + 1) * 512],
                        ones_row[:],
                        recip[:, u * 512:(u + 1) * 512],
                        start=True, stop=True,
                    )
                rb_s = smallp.tile([P, N], FP32, tag="rb")
                nc.scalar.copy(rb_s[:], prb[:, :])
                # normalized O^T in bf16
                otn = smallp.tile([P, N], BF16, tag="otn")
                nc.vector.tensor_mul(otn[:], pot[:, :], rb_s[:])
                return otn

            def outproj(h, otn):
                for ob in range(DIM // P):
                    po = psum.tile([P, N], FP32, tag="sc", bufs=2)
                    lhs = wo_s[:, h, ob * P:(ob + 1) * P]
                    for u in range(N // 512):
                        nc.tensor.matmul(
                            po[:, u * 512:(u + 1) * 512],
                            lhs,
                            otn[:, u * 512:(u + 1) * 512],
                            start=True,
                            stop=True,
                        )
                    o_t = outp.tile([P, N], FP32, tag="o")
                    if ob % 2 == 0:
                        nc.scalar.copy(o_t[:], po[:, :])
                    else:
                        nc.vector.tensor_copy(o_t[:], po[:, :])
                    nc.sync.dma_start(pouts[h][ob * P:(ob + 1) * P, :], o_t[:])

            otn0 = attention(0, st0)
            outproj(0, otn0)
            otn1 = attention(1)
            outproj(1, otn1)

          with (tc.For_i(0, reps, 1) if reps > 1 else _nullctx()):
            _body()

    nc.compile()
    return nc


def _prep_in_maps(x, context, doc_similarities, beta, Wq, Wkv, Wout):
    """Shard + lay out per-core inputs (host-side transposes/casts)."""
    Wk, Wv = Wkv[:DIM], Wkv[DIM:]
    in_maps = []
    for c in range(8):
        b, hg = c // HG, c % HG
        rows = slice(NHPC * D * hg, NHPC * D * (hg + 1))
        simv = (np.repeat(np.asarray(doc_similarities[b], np.float32), CL)
                * np.float32(beta))                       # [J]
        simb = np.ascontiguousarray(simv.reshape(JT, P).T)  # [128, 32]
        in_maps.append({
            "xT": np.ascontiguousarray(np.asarray(x[b], np.float32).T).astype(NPBF16),
            "ctxT": np.ascontiguousarray(
                np.asarray(context[b], np.float32).reshape(J, DIM).T).astype(NPBF16),
            "wqT": np.ascontiguousarray(np.asarray(Wq, np.float32)[rows].T).astype(NPBF16),
            "wkT": np.ascontiguousarray(np.asarray(Wk, np.float32)[rows].T).astype(NPBF16),
            "wvT": np.ascontiguousarray(np.asarray(Wv, np.float32)[rows].T).astype(NPBF16),
            "woutT": np.ascontiguousarray(
                np.asarray(Wout, np.float32)[:, rows].T).astype(NPBF16),
            "simb": simb.astype(np.float32),
        })
    return in_maps


def kernel(x, context, doc_similarities, mask, context_mask, Wq, Wkv, beta,
           Wout, bout, **_unused):
    if "nc" not in _CACHE:
        _CACHE["nc"] = _build_nc()
    nc = _CACHE["nc"]

    in_maps = _prep_in_maps(x, context, doc_similarities, beta, Wq, Wkv, Wout)
    res = run_bass_kernel_spmd(nc, in_maps, core_ids=list(range(8)))
    _CACHE["last_result"] = res

    bout32 = np.asarray(bout, np.float32)
    out = np.zeros((B, N, DIM), np.float32)
    for c in range(8):
        b = c // HG
        for h in range(NHPC):
            out[b] += res.results[c][f"pout{h}"].T
    out += bout32[None, None, :]
    return out



# revision 2
# speedup vs baseline: 1.1929x; 1.1929x over previous
"""Trainium2 Bass kernel for nn_CrossAttention (b=2, n=1024, dim=1024, h=8,
m=4, cl=1024).

Sharding: 8 cores = batch(2) x head-groups(4, 2 heads each).
Each core computes q/k/v projections for its 2 heads, streaming softmax
attention in transposed layout (S^T = [j, i]), and per-head partial output
projections out^T.  Host sums the 8 partials per batch and adds bout.

v2 schedule: one flattened stream of 64 attention steps (2 heads staggered
by 4 j-tiles), with k/v/q production chunks interleaved as PE filler at a
uniform cadence so every engine stays balanced.  All on-chip data is fp16
(better mantissa than bf16, 2x DVE mode for the softmax-denominator
accumulation, and 1 cycle/row for the fp16 rowsum/broadcast matmuls).

Layout notes (PE matmul computes lhsT.T @ rhs, lhsT stationary [K<=128, M<=128]):
  qT[dh, i]  = WqT-blocks.T @ xT          (lhsT = WqT [k, dh], rhs = xT [k, i])
  kT[dh, j]  = WkT-blocks.T @ ctxT        (lhsT = WkT [k, dh], rhs = ctxT [k, j])
  v[j, dv]   = ctxT-blocks.T @ WvT        (lhsT = ctxT [k, j], rhs = WvT [k, dv])
  S^T[j, i]  = kT-block.T @ qT            (lhsT = kT [d, j],  rhs = qT [d, i])
  P^T        = exp(scale*S^T + sim[j])    (ScalarE, per-partition bias)
  O^T[dv, i] = sum_j v-block.T @ P^T      (lhsT = v [j, dv],  rhs = P^T [j, i])
  racc[r,i]  = sum_jt P^T tiles           (DVE fp16 2x mode)
  rowsum[1,i]= ones.T @ racc              (fp16 matmul, 1 cyc/row)
  out^T[o,i] = WoutT-block.T @ (O^T/rowsum)
"""

import numpy as np

import concourse.bass as bass
import concourse.tile as tile
import concourse.mybir as mybir
from concourse import bacc
from concourse.bass_utils import run_bass_kernel_spmd

FP16 = mybir.dt.float16
FP32 = mybir.dt.float32
NPFP16 = np.float16

# problem sizes (hardcoded)
B, N, DIM, H, D = 2, 1024, 1024, 8, 128
M, CL = 4, 1024
J = M * CL            # 4096
NHPC = 2              # heads per core
HG = H // NHPC        # head groups = 4
SCALE = DIM ** -0.5   # 1/32
KT = DIM // 128       # 8 k-slices
JT = J // 128         # 32 j-tiles
P = 128
LAG = 16              # h1 attention stagger (j-tiles) behind h0

_CACHE = {}


class _nullctx:
    def __enter__(self):
        return None

    def __exit__(self, *a):
        return False


def _build_nc(reps=1):
    nc = bacc.Bacc("TRN2", target_bir_lowering=False, debug=False)

    xT = nc.dram_tensor("xT", [DIM, N], FP16, kind="ExternalInput").ap()
    ctxT = nc.dram_tensor("ctxT", [DIM, J], FP16, kind="ExternalInput").ap()
    wqT = nc.dram_tensor("wqT", [DIM, NHPC * D], FP16, kind="ExternalInput").ap()
    wkT = nc.dram_tensor("wkT", [DIM, NHPC * D], FP16, kind="ExternalInput").ap()
    wvT = nc.dram_tensor("wvT", [DIM, NHPC * D], FP16, kind="ExternalInput").ap()
    woutT = nc.dram_tensor("woutT", [NHPC * D, DIM], FP16, kind="ExternalInput").ap()
    simb = nc.dram_tensor("simb", [P, JT], FP32, kind="ExternalInput").ap()
    pouts = [nc.dram_tensor(f"pout{h}", [DIM, N], FP16, kind="ExternalOutput").ap()
             for h in range(NHPC)]

    with tile.TileContext(nc) as tc:
        with (
            tc.tile_pool(name="const", bufs=1) as constp,
            tc.tile_pool(name="weights", bufs=1) as wp,
            tc.tile_pool(name="acts", bufs=1) as ap_,
            tc.tile_pool(name="work", bufs=1) as workp,
            tc.tile_pool(name="ppool", bufs=3) as ppool,
            tc.tile_pool(name="small", bufs=2) as smallp,
            tc.tile_pool(name="outs", bufs=4) as outp,
            tc.tile_pool(name="psum", bufs=1, space="PSUM") as psum,
        ):
          def _body():
            # ---- constants ----
            ones_col = constp.tile([P, 1], FP16)
            nc.any.memset(ones_col[:], 1.0)
            ones_row = constp.tile([1, P], FP16)
            nc.any.memset(ones_row[:], 1.0)
            simb_s = constp.tile([P, JT], FP32)
            nc.sync.dma_start(simb_s[:], simb[:, :])

            # ---- DMA, priority order ----
            wq_s = wp.tile([P, KT, NHPC * D], FP16)
            wqr = wqT.rearrange("(a p) m -> p a m", p=P)
            nc.sync.dma_start(wq_s[:, 0:4, :], wqr[:, 0:4, :])
            nc.sync.dma_start(wq_s[:, 4:8, :], wqr[:, 4:8, :])
            xT_s = ap_.tile([P, KT, N], FP16)
            xTr = xT.rearrange("(a p) m -> p a m", p=P)
            for xk in range(4):
                nc.sync.dma_start(xT_s[:, 2 * xk:2 * xk + 2, 0:512],
                                  xTr[:, 2 * xk:2 * xk + 2, 0:512])
            for xk in range(4):
                nc.sync.dma_start(xT_s[:, 2 * xk:2 * xk + 2, 512:1024],
                                  xTr[:, 2 * xk:2 * xk + 2, 512:1024])
            wk_s = wp.tile([P, KT, NHPC * D], FP16)
            nc.sync.dma_start(wk_s[:], wkT.rearrange("(a p) m -> p a m", p=P))
            wv_s = wp.tile([P, KT, NHPC * D], FP16)
            nc.sync.dma_start(wv_s[:], wvT.rearrange("(a p) m -> p a m", p=P))
            ctx_s = ap_.tile([P, KT, J], FP16)
            JQW = 1024  # ctx streamed per quarter
            for jq in range(4):
                for k in range(KT):
                    nc.sync.dma_start(
                        ctx_s[:, k, jq * JQW:(jq + 1) * JQW],
                        ctxT[k * P:(k + 1) * P, jq * JQW:(jq + 1) * JQW],
                    )
            wo_s = wp.tile([P, NHPC, DIM], FP16)
            nc.sync.dma_start(wo_s[:], woutT.rearrange("(a p) m -> p a m", p=P))

            # ---- persistent work tiles ----
            qT_s = workp.tile([P, NHPC, N], FP16)
            kT_s = workp.tile([P, NHPC, J], FP16)
            v_s = workp.tile([P, JT, NHPC * D], FP16)
            raccs = [smallp.tile([P, N], FP16, tag=f"racc{h}", bufs=1,
                                 name=f"racc{h}") for h in range(NHPC)]
            otns = [smallp.tile([P, N], FP16, tag=f"otn{h}", bufs=1,
                                name=f"otn{h}") for h in range(NHPC)]
            pots = [psum.tile([P, N], FP32, tag="ot", bufs=2,
                              name=f"pot{h}") for h in range(NHPC)]

            # ---- production units ----
            def q_chunk(h, u):
                pq = psum.tile([P, 512], FP32, tag="pjv", bufs=1)
                for k in range(KT):
                    nc.tensor.matmul(
                        pq[:, :],
                        wq_s[:, k, h * D:(h + 1) * D],
                        xT_s[:, k, u * 512:(u + 1) * 512],
                        start=(k == 0),
                        stop=(k == KT - 1),
                    )
                nc.vector.tensor_copy(qT_s[:, h, u * 512:(u + 1) * 512], pq[:, :])

            def kt_chunk(h, e):
                """kT 512-wide chunk e for head h."""
                pk = psum.tile([P, 512], FP32, tag="pjv", bufs=1)
                for k in range(KT):
                    nc.tensor.matmul(
                        pk[:, :],
                        wk_s[:, k, h * D:(h + 1) * D],
                        ctx_s[:, k, e * 512:(e + 1) * 512],
                        start=(k == 0),
                        stop=(k == KT - 1),
                    )
                nc.vector.tensor_copy(kT_s[:, h, e * 512:(e + 1) * 512], pk[:, :])

            _vctr = [0]

            def v_chunk(jt):
                pv = psum.tile([P, NHPC * D], FP32, tag="pjv", bufs=1)
                for k in range(KT):
                    nc.tensor.matmul(
                        pv[:, :],
                        ctx_s[:, k, jt * P:(jt + 1) * P],
                        wv_s[:, k, :],
                        start=(k == 0),
                        stop=(k == KT - 1),
                    )
                # alternate ACT / DVE (GPSIMD cannot read PSUM)
                if _vctr[0] % 2 == 0:
                    nc.scalar.copy(v_s[:, jt, :], pv[:, :])
                else:
                    nc.vector.tensor_copy(v_s[:, jt, :], pv[:, :])
                _vctr[0] += 1

            # ---- attention step, software-pipelined in two halves ----
            # S-half: S^T matmuls + exp into p_t; O-half (emitted one stream
            # position later): O^T accumulation + denominator add.
            pending = []

            def attn_S(h, jt):
                kblk = kT_s[:, h, jt * P:(jt + 1) * P]
                p_t = ppool.tile([P, N], FP16, tag="p")
                for u in range(2):
                    ps_ = psum.tile([P, 512], FP32, tag="ps", bufs=3)
                    nc.tensor.matmul(
                        ps_[:, :], kblk,
                        qT_s[:, h, u * 512:(u + 1) * 512],
                        start=True, stop=True,
                    )
                    nc.scalar.activation(
                        p_t[:, u * 512:(u + 1) * 512], ps_[:, :],
                        mybir.ActivationFunctionType.Exp,
                        bias=simb_s[:, jt:jt + 1],
                        scale=float(SCALE),
                    )
                pending.append((h, jt, p_t))

            def attn_O():
                h, jt, p_t = pending.pop(0)
                pot = pots[h]
                vblk = v_s[:, jt, h * D:(h + 1) * D]
                for u in range(2):
                    nc.tensor.matmul(
                        pot[:, u * 512:(u + 1) * 512],
                        vblk,
                        p_t[:, u * 512:(u + 1) * 512],
                        start=(jt == 0),
                        stop=(jt == JT - 1),
                    )
                # softmax denominator partial sums, fp16 2x DVE mode
                if jt == 0:
                    nc.vector.tensor_copy(raccs[h][:], p_t[:])
                else:
                    nc.vector.tensor_add(raccs[h][:], raccs[h][:], p_t[:])

            # ---- normalize: otn[:, u-half] = pot / colsum(racc) ----
            def normalize_chain(h, u, fill=None):
                racc, pot, otn = raccs[h], pots[h], otns[h]
                sl = slice(u * 512, (u + 1) * 512)
                prs = psum.tile([P, 512], FP32, tag="pjv", bufs=1)
                nc.tensor.matmul(prs[:1, :], ones_col[:], racc[:, sl],
                                 start=True, stop=True)
                rs = smallp.tile([1, 512], FP16, tag="rs")
                nc.vector.tensor_copy(rs[:], prs[:1, :])
                if fill:
                    fill.pop(0)()
                prb = psum.tile([P, 512], FP32, tag="pjv", bufs=1)
                nc.tensor.matmul(prb[:, :], ones_row[:], rs[:],
                                 start=True, stop=True)
                rb = smallp.tile([P, 512], FP32, tag="rb")
                nc.vector.reciprocal(rb[:], prb[:, :])
                nc.vector.tensor_mul(otn[:, sl], pot[:, sl], rb[:])

            # ---- output projection chunk: pout[h][ob, u-half] ----
            _octr = [0]
            _ocur = {}
            _odone = set()

            def outproj_chunk(h, ob, u):
                po = psum.tile([P, 512], FP32, tag="ps", bufs=3)
                nc.tensor.matmul(
                    po[:, :],
                    wo_s[:, h, ob * P:(ob + 1) * P],
                    otns[h][:, u * 512:(u + 1) * 512],
                    start=True, stop=True,
                )
                key = (h, ob)
                if key not in _ocur:
                    _ocur[key] = outp.tile([P, N], FP16, tag="o", name=f"o_{h}_{ob}")
                o_t = _ocur[key]
                if _octr[0] % 2 == 0:
                    nc.vector.tensor_copy(o_t[:, u * 512:(u + 1) * 512], po[:, :])
                else:
                    nc.scalar.copy(o_t[:, u * 512:(u + 1) * 512], po[:, :])
                _octr[0] += 1
                if key in _odone:
                    nc.sync.dma_start(pouts[h][ob * P:(ob + 1) * P, :], o_t[:])
                    del _ocur[key]
                else:
                    _odone.add(key)

            # ================= flattened stream =================
            # q-projection first (only needs x + wq)
            q_chunk(0, 0)
            q_chunk(1, 0)
            q_chunk(0, 1)
            q_chunk(1, 1)

            # production queue with "needed by stream position" keys and
            # PE-cost (us) for metering.
            # kt(h0,e) needed at s=4e; kt(h1,e) at s=4e+LAG; v(jt) at s=jt
            prodq = []
            for e in range(8):
                prodq.append((4 * e, 1.7, lambda e=e: kt_chunk(0, e)))
                prodq.append((4 * e + LAG, 1.7, lambda e=e: kt_chunk(1, e)))
            for jt in range(JT):
                prodq.append((jt, 0.85, lambda jt=jt: v_chunk(jt)))
            prodq.sort(key=lambda t: t[0])
            total_pe = sum(t[1] for t in prodq)

            h0_po = [(lambda i=i: outproj_chunk(0, i // 2, i % 2))
                     for i in range(16)]
            NPOS = JT + LAG  # 48 stream positions
            RATE = total_pe / (NPOS - 4)  # drain slightly before the end
            cum = [0.0]

            def pop_prod(s, hard):
                """Emit one production unit if due (hard: needed now;
                soft: metered rate allows and within lookahead)."""
                if not prodq:
                    return False
                nb, cost, fn = prodq[0]
                due = nb <= s + 1 if hard else (
                    cum[0] + cost <= RATE * (s + 1) and nb <= s + 10)
                if due:
                    prodq.pop(0)
                    cum[0] += cost
                    fn()
                    return True
                return False

            for s in range(NPOS):
                pop_prod(s, hard=True)
                if s < JT:
                    attn_S(0, s)
                if not pop_prod(s, hard=True):
                    pop_prod(s, hard=False)
                if LAG <= s:
                    attn_S(1, s - LAG)
                if not pop_prod(s, hard=True):
                    pop_prod(s, hard=False)
                while len(pending) > 2:
                    attn_O()
                pop_prod(s, hard=False)
                if s == JT + 1:
                    normalize_chain(0, 0)
                if s == JT + 2:
                    normalize_chain(0, 1)
                if s >= JT + 3 and len(h0_po) > 2:
                    h0_po.pop(0)()
            while pending:
                attn_O()
            # tail: h1 normalize (u-pipelined) + outproj, with h0's last
            # outproj chunks as PE filler inside the chain latency.
            normalize_chain(1, 0, fill=h0_po)
            if h0_po:
                h0_po.pop(0)()
            normalize_chain(1, 1, fill=h0_po)
            # u0 chunks lead so they overlap the u1 normalize chain; the
            # sliding (ob-3, u1) interleave keeps <= 4 live o_t tiles.
            order = []
            for ob in range(8):
                order.append((ob, 0))
                if ob >= 3:
                    order.append((ob - 3, 1))
            order += [(ob, 1) for ob in range(5, 8)]
            for ob, u in order:
                if h0_po:
                    h0_po.pop(0)()
                outproj_chunk(1, ob, u)

          with (tc.For_i(0, reps, 1) if reps > 1 else _nullctx()):
            _body()

    nc.compile()
    return nc


def _prep_in_maps(x, context, doc_similarities, beta, Wq, Wkv, Wout):
    """Shard + lay out per-core inputs (host-side transposes/casts)."""
    Wk, Wv = Wkv[:DIM], Wkv[DIM:]
    in_maps = []
    for c in range(8):
        b, hg = c // HG, c % HG
        rows = slice(NHPC * D * hg, NHPC * D * (hg + 1))
        simv = (np.repeat(np.asarray(doc_similarities[b], np.float32), CL)
                * np.float32(beta))                       # [J]
        simb = np.ascontiguousarray(simv.reshape(JT, P).T)  # [128, 32]
        in_maps.append({
            "xT": np.ascontiguousarray(np.asarray(x[b], np.float32).T).astype(NPFP16),
            "ctxT": np.ascontiguousarray(
                np.asarray(context[b], np.float32).reshape(J, DIM).T).astype(NPFP16),
            "wqT": np.ascontiguousarray(np.asarray(Wq, np.float32)[rows].T).astype(NPFP16),
            "wkT": np.ascontiguousarray(np.asarray(Wk, np.float32)[rows].T).astype(NPFP16),
            "wvT": np.ascontiguousarray(np.asarray(Wv, np.float32)[rows].T).astype(NPFP16),
            "woutT": np.ascontiguousarray(
                np.asarray(Wout, np.float32)[:, rows].T).astype(NPFP16),
            "simb": simb.astype(np.float32),
        })
    return in_maps


def kernel(x, context, doc_similarities, mask, context_mask, Wq, Wkv, beta,
           Wout, bout, **_unused):
    if "nc" not in _CACHE:
        _CACHE["nc"] = _build_nc()
    nc = _CACHE["nc"]

    in_maps = _prep_in_maps(x, context, doc_similarities, beta, Wq, Wkv, Wout)
    res = run_bass_kernel_spmd(nc, in_maps, core_ids=list(range(8)))
    _CACHE["last_result"] = res

    bout32 = np.asarray(bout, np.float32)
    out = np.zeros((B, N, DIM), np.float32)
    for c in range(8):
        b = c // HG
        for h in range(NHPC):
            out[b] += res.results[c][f"pout{h}"].T.astype(np.float32)
    out += bout32[None, None, :]
    return out
